# revision 19
# baseline (speedup 1.0000x reference)
"""TRN2 Bass/Tile attention kernel v8 (B=4, H=16, S=2048, D=64, fp32).

Entry point: kernel(q, k, v) -> out, all full-shape [4, 16, 2048, 64]
fp32. Sharding: batch*heads = 64 head-slices, 8 per NeuronCore (head
parallel, no cross-core communication); same NEFF on every core via
run_bass_kernel_spmd.

Per-core algorithm: S^T-formulation attention (j-chunk-stationary QK,
V-stationary PV with a ones-column accumulating the softmax
denominator). The device is reduced to its irreducible QK -> exp -> PV
pipeline; everything layout-ish lives on the host:

- Inputs are packed HOST-side into the exact SBUF layouts the matmuls
  need (bf16, d-major transposed q/k with the q hi-half duplicated for
  row-tiled QK streams, chunk-major v with the ones column baked in).
  This removes all per-head input transposes from the PE, their PSUM
  bank, and every DVE convert/copy on the input path, and cuts DMA
  bytes ~35% vs f32 loads.
- The OUTPUT leaves the device d-major and unnormalized ([heads, 65,
  S]; row 64 = denominator); the host divides and transposes back.
  This deletes the whole PE transpose-back epilogue and its PSUM bank,
  freeing the banks that make the FD-1536 exp tiling below possible.
- ScalarE is the bottleneck: exp of all S^2 logits = 33.5M elem/core
  at 1 elem/cycle/lane @1.2GHz = 218us data floor. Its ~220-cycle
  per-instruction overhead is amortized with FD-1536 activations: two
  persistent 3-bank PSUM slots, each filled by three FD-512 QK
  matmuls and drained by ONE activation (176 activations/rep instead
  of 256; ScalarE busy 251us vs 266us).
- Software pipelining per unit: QK matmuls for unit n, then the exp
  for unit n-1, then PV matmuls for unit n-2's half-ticks. QK always
  retires a full exp-window before its own exp issues, and PVs are
  only emitted once their activation has retired, so neither ever
  head-blocks the PE queue into a ScalarE stall. (Tile collapses
  dependencies into per-engine counter waits, so an exp's gate covers
  every PE instruction emitted before it - emission order IS the
  dependency structure.)
- PSUM: 2x3 banks QK-logit slots + 2x1 bank double-buffered PV
  accumulator. Loads are issued two heads ahead into 4 rotating
  SBUF slots (8 heads/rep: the 2-ahead load never collides with a
  live slot, including at the For_i wrap).

Measured: 269us/rep (v7, 256 activations) -> ~255us/rep (v8) vs the
324-361us v2 baseline; rel err 5.0e-3 against the fp32 reference
(gate 2e-2). Simulated ScalarE occupancy ~95%.

This container's walrus build rejects sync waits on Drain instructions
and allows at most one sync wait on any other instruction, while Tile
freely attaches several; _patch_tile_framework() + _split_sync_waits()
below rework the exit barrier and hoist excess waits onto injected NOPs.
"""
import sys

if '/opt/trn_rl_repo' not in sys.path:
    sys.path.insert(0, '/opt/trn_rl_repo')

import numpy as np
import ml_dtypes

import concourse.bass as bass
import concourse.tile as tile
from concourse import mybir
from concourse.vector_clock import ScopedClock

F32 = mybir.dt.float32
F16 = mybir.dt.bfloat16   # PE-native 16-bit dtype (78.6 TF/s); fp16 is not
EXP = mybir.ActivationFunctionType.Exp
BF16 = ml_dtypes.bfloat16

B, H, S, D = 4, 16, 2048, 64
N_CORES = 8
HEADS_PER_CORE = B * H // N_CORES


# ---------------------------------------------------------------------------
# Walrus compatibility patches (same as v1)
# ---------------------------------------------------------------------------
_patched = False
_split_counter = [0]


def _patched_multi_engine_barrier(self, engines):
    for e in engines:
        self.engines[e].drain(fusable=False)
    for inst in self._sem_only_all_engine_barrier_insts(f"aeb{self.next_id()}"):
        self.engines[inst.engine].add_instruction(inst)


def _patched_drain_and_barrier(self, tick_clock, wait_clock):
    nop_inst = self.nc.sync.nop(nofuse=True, hint="tile_exit_wait")
    wait_clock.add_sem_waits(
        nop_inst.ins, ScopedClock({None: tick_clock.global_clock})
    )
    self.nc.sync.drain()
    self.nc.all_engine_barrier()
    assert self.sems is not None
    popped = self.nc._tile_sem_poison_stack.pop()
    assert popped is self._sem_poison
    self.nc.clear_and_free_semaphores(list(self.sems.allocated().values()))
    self.nc.all_engine_barrier()


def _patch_tile_framework():
    global _patched
    if _patched:
        return
    bass.Bass.multi_engine_barrier = _patched_multi_engine_barrier
    tile.TileContext._drain_and_barrier = _patched_drain_and_barrier
    _patched = True


def _split_sync_waits(nc):
    """No instruction may carry more than the walrus-supported number of
    sync waits (0 for Drain, 1 otherwise); hoist the rest onto NOPs."""
    for f in nc.m.functions:
        for bb in f.blocks:
            insts = bb.instructions
            if not any(
                i.sync_info is not None
                and len(i.sync_info.on_wait) > (0 if i.opcode == "Drain" else 1)
                for i in insts
            ):
                continue
            out = []
            for inst in insts:
                si = inst.sync_info
                limit = 0 if inst.opcode == "Drain" else 1
                if si is not None and len(si.on_wait) > limit:
                    waits = list(si.on_wait)
                    keep, extra = waits[:limit], waits[limit:]
                    for w in extra:
                        _split_counter[0] += 1
                        nop = mybir.InstNoOp(
                            name=f"waitsplit-{_split_counter[0]}", ins=[], outs=[]
                        )
                        nop.engine = inst.engine
                        nop.sync_info = mybir.SyncInfo(on_wait=[w], on_update=[])
                        out.append(nop)
                    inst.sync_info = mybir.SyncInfo(
                        on_wait=keep, on_update=list(si.on_update)
                    )
                out.append(inst)
            bb.instructions = out


# ---------------------------------------------------------------------------
# Host-side input packing
# ---------------------------------------------------------------------------
def prep_inputs(q, k, v):
    """Pack full-shape fp32 q/k/v into per-core bf16 SBUF-layout tensors.

    qT: [hpc, 128, S]      qT[h, 64a+d, i]        = q[h, i, d]   (hi-dup)
    kT: [hpc, 128, S//2]   kT[h, 64a+d, 128cc+p]  = k[h, 256cc+128a+p, d]
    vt: [hpc, 128, NJ, 65] vt[h, p, c, d]         = v[h, 128c+p, d]; [...,64]=1
    """
    NJ = S // 128
    q = np.ascontiguousarray(np.asarray(q, np.float32)).reshape(B * H, S, D)
    k = np.ascontiguousarray(np.asarray(k, np.float32)).reshape(B * H, S, D)
    v = np.ascontiguousarray(np.asarray(v, np.float32)).reshape(B * H, S, D)

    qt = q.transpose(0, 2, 1)                                   # [64h?, D, S]
    qT = np.concatenate([qt, qt], axis=1).astype(BF16)          # [_, 128, S]

    kr = k.reshape(B * H, NJ // 2, 2, 128, D)                   # [_,cc,a,p,d]
    kT = np.ascontiguousarray(
        kr.transpose(0, 2, 4, 1, 3)                             # [_,a,d,cc,p]
    ).reshape(B * H, 128, (NJ // 2) * 128).astype(BF16)

    vr = v.reshape(B * H, NJ, 128, D).transpose(0, 2, 1, 3)     # [_,p,c,d]
    vt = np.empty((B * H, 128, NJ, D + 1), np.float32)
    vt[:, :, :, 0:D] = vr
    vt[:, :, :, D] = 1.0
    vt = vt.astype(BF16)

    hpc = HEADS_PER_CORE
    return [
        {"qT": qT[i * hpc:(i + 1) * hpc],
         "kT": kT[i * hpc:(i + 1) * hpc],
         "vt": vt[i * hpc:(i + 1) * hpc]}
        for i in range(N_CORES)
    ]


# ---------------------------------------------------------------------------
# Kernel builder
# ---------------------------------------------------------------------------
def build_nc(heads=HEADS_PER_CORE, s=S, reps=1):
    NJ = s // 128            # 16 j (k-row) chunks of 128
    NP = NJ // 2             # 8 chunk pairs
    IG = 512                 # i (q-row) group width
    NG = s // IG             # 4 groups
    NT = IG // 128           # 4 output tiles per group
    TICKS = NG * NP          # 32 ticks per head
    scale = D ** -0.5

    nc = bass.Bass(target_bir_lowering=False)
    qT_d = nc.dram_tensor("qT", [heads, 128, s], F16, kind="ExternalInput")
    kT_d = nc.dram_tensor("kT", [heads, 128, s // 2], F16, kind="ExternalInput")
    vt_d = nc.dram_tensor("vt", [heads, 128, NJ, D + 1], F16,
                          kind="ExternalInput")
    # Output is written d-major and UNNORMALIZED (65th row = softmax
    # denominator); the host divides and transposes back.
    o_d = nc.dram_tensor("o", [heads, D + 1, s], F32, kind="ExternalOutput")

    with tile.TileContext(nc) as tc:
        with (
            tc.tile_pool(name="perst", bufs=1) as perst,
            tc.tile_pool(name="ex1", bufs=4) as ex1,
            tc.tile_pool(name="osb", bufs=3) as osb,
            tc.tile_pool(name="qk3", bufs=1, space="PSUM") as qk3,
            tc.tile_pool(name="pvps", bufs=2, space="PSUM") as pvps,
        ):
            # Two persistent 3-bank QK-logit slots. Each holds up to three
            # 512-col "half-tick" thirds that are consumed by ONE FD-1536
            # activation, amortizing the ~220-cycle per-activation overhead
            # (176 instead of 256 activations per rep).
            pss = [qk3.tile([128, 3, IG], F32, tag=f"ps{i}", name=f"ps{i}")
                   for i in range(2)]

            # Persistent quad-buffered per-head tensors (slot h%4: with 8
            # heads per rep the 2-ahead loads never target a slot the
            # current or next head is reading, including at the wrap).
            qT = [perst.tile([128, s], F16, tag=f"qT{i}", name=f"qT{i}")
                  for i in range(4)]
            kT = [perst.tile([128, s // 2], F16, tag=f"kT{i}", name=f"kT{i}")
                  for i in range(4)]
            vt = [perst.tile([128, NJ, D + 1], F16, tag=f"vt{i}", name=f"vt{i}")
                  for i in range(4)]

            def load_ops(h):
                """DMA the pre-packed head h into slot h%3 (emitted two
                heads early; contiguous bf16 reads)."""
                sl = h % 4

                def load():
                    nc.sync.dma_start(out=qT[sl], in_=qT_d[h])
                    nc.sync.dma_start(out=kT[sl], in_=kT_d[h])
                    nc.sync.dma_start(out=vt[sl], in_=vt_d[h])
                return [(False, load)]

            def epi_ops(h, g, pv):
                """Epilogue closures for (head h, group g): evacuate pv to
                SBUF, DMA it out d-major (normalization happens on host)."""
                og = osb.tile([D + 1, IG], F32, tag="og")
                ops = []

                def evac():
                    nc.vector.tensor_copy(og, pv)
                ops.append((False, evac))

                def out_dma():
                    nc.sync.dma_start(
                        out=o_d[h, :, g * IG:(g + 1) * IG], in_=og)
                ops.append((False, out_dma))
                return ops

            # Per-head schedule: 64 half-ticks (g-major, chunk-minor), in
            # units of 3 (FD-1536 exp) with two 2-unit tails (FD-1024).
            UNIT_SIZES = [3] * 20 + [2, 2]

            def body():
                queue = []       # pending sprinkle closures [(is_pe, fn)]
                pv_tiles = {}    # (h, g) -> psum tile
                exp_pend = None  # (slot, size, [(h, g, c), ...]) awaiting exp
                pv_ready = []    # [(seq, (h, g, c), et_slice)] awaiting PV
                useq = [0]       # global unit counter

                def pop_sprinkles():
                    while queue:
                        queue.pop(0)[1]()

                def do_exp(slot, size, hgcs):
                    et = ex1.tile([128, 3, IG], F16, tag="et1")
                    nc.scalar.activation(
                        et[:, 0:size, :], pss[slot][:, 0:size, :],
                        EXP, scale=scale)
                    for idx, hgc in enumerate(hgcs):
                        pv_ready.append((useq[0], hgc, et[:, idx, :]))

                def emit_pv(hgc, et_slice):
                    h, g, c = hgc
                    if c == 0:
                        pv_tiles[(h, g)] = pvps.tile(
                            [D + 1, IG], F32, tag="pv", name="pv")
                    pv = pv_tiles[(h, g)]
                    nc.tensor.matmul(
                        pv, vt[h % 4][:, c, :], et_slice,
                        start=(c == 0), stop=(c == NJ - 1))
                    if c == NJ - 1:
                        queue.extend(epi_ops(h, g, pv))
                        del pv_tiles[(h, g)]

                for h in range(heads):
                    hts = [(g, c) for g in range(NG) for c in range(NJ)]
                    sl = h % 4
                    pos = 0
                    for u, size in enumerate(UNIT_SIZES):
                        if u == 0:
                            queue.extend(load_ops((h + 2) % heads))
                        slot = useq[0] % 2
                        cur = hts[pos:pos + size]
                        pos += size

                        # QK matmuls, one per half-tick, into slot thirds.
                        # 2-slot rotation: this unit's QKs wait only the exp
                        # issued two units back, whose window has passed.
                        for idx, (g, c) in enumerate(cur):
                            a = 64 * (c % 2)
                            nc.tensor.matmul(
                                pss[slot][:, idx, :],
                                kT[sl][a:a + 64,
                                       (c // 2) * 128:(c // 2) * 128 + 128],
                                qT[sl][a:a + 64, g * IG:(g + 1) * IG],
                                start=True, stop=True,
                                tile_position=(a, 0))

                        if exp_pend is not None:
                            do_exp(*exp_pend)   # exp for the previous unit
                        exp_pend = (slot, size, [(h, g, c) for g, c in cur])

                        pop_sprinkles()

                        # PV for half-ticks whose exp was issued in an
                        # earlier unit (keeps waiting PVs off the PE queue
                        # until their activation has retired).
                        while pv_ready and pv_ready[0][0] < useq[0]:
                            _, hgc, ets = pv_ready.pop(0)
                            emit_pv(hgc, ets)
                        useq[0] += 1

                # drain: last unit's exp, remaining PVs, sprinkles
                do_exp(*exp_pend)
                exp_pend = None
                while pv_ready:
                    _, hgc, ets = pv_ready.pop(0)
                    emit_pv(hgc, ets)
                while queue:
                    queue.pop(0)[1]()

            # Prologue: load heads 0+1 (one-time; outside For_i).
            for _, fn in load_ops(0) + load_ops(1):
                fn()

            if reps == 1:
                body()
            else:
                with tc.For_i(0, reps, 1):
                    body()

    _split_sync_waits(nc)
    return nc


_cached_nc = None


def _get_nc():
    global _cached_nc
    if _cached_nc is None:
        _patch_tile_framework()
        _cached_nc = build_nc()
    return _cached_nc


def kernel(q, k, v):
    """Full-shape attention: q/k/v [4, 16, 2048, 64] fp32 -> same shape."""
    from concourse.bass_utils import run_bass_kernel_spmd

    nc = _get_nc()
    in_maps = prep_inputs(q, k, v)
    res = run_bass_kernel_spmd(nc, in_maps, core_ids=list(range(N_CORES)))
    out = np.concatenate([res.results[i]["o"] for i in range(N_CORES)], axis=0)
    # out: [B*H, D+1, S] unnormalized, d-major; row D is the softmax denom.
    out = out[:, 0:D, :] / out[:, D:D + 1, :]
    return np.ascontiguousarray(out.transpose(0, 2, 1)).reshape(B, H, S, D)


# revision 21
# speedup vs baseline: 1.0183x; 1.0183x over previous
"""TRN2 Bass/Tile attention kernel v8 (B=4, H=16, S=2048, D=64, fp32).

Entry point: kernel(q, k, v) -> out, all full-shape [4, 16, 2048, 64]
fp32. Sharding: batch*heads = 64 head-slices, 8 per NeuronCore (head
parallel, no cross-core communication); same NEFF on every core via
run_bass_kernel_spmd.

Per-core algorithm: S^T-formulation attention (j-chunk-stationary QK,
V-stationary PV with a ones-column accumulating the softmax
denominator). The device is reduced to its irreducible QK -> exp -> PV
pipeline; everything layout-ish lives on the host:

- Inputs are packed HOST-side into the exact SBUF layouts the matmuls
  need (bf16, d-major transposed q/k with the q hi-half duplicated for
  row-tiled QK streams, chunk-major v with the ones column baked in).
  This removes all per-head input transposes from the PE, their PSUM
  bank, and every DVE convert/copy on the input path, and cuts DMA
  bytes ~35% vs f32 loads.
- The OUTPUT leaves the device d-major and unnormalized ([heads, 65,
  S]; row 64 = denominator); the host divides and transposes back.
  This deletes the whole PE transpose-back epilogue and its PSUM bank,
  freeing the banks that make the FD-1536 exp tiling below possible.
- ScalarE is the bottleneck: exp of all S^2 logits = 33.5M elem/core
  at 1 elem/cycle/lane @1.2GHz = 218us data floor. Its ~220-cycle
  per-instruction overhead is amortized with FD-1536 activations: two
  persistent 3-bank PSUM slots, each filled by three FD-512 QK
  matmuls and drained by ONE activation (176 activations/rep instead
  of 256; ScalarE busy 251us vs 266us).
- Software pipelining per unit: QK matmuls for unit n, then the exp
  for unit n-1, then PV matmuls for unit n-2's half-ticks. QK always
  retires a full exp-window before its own exp issues, and PVs are
  only emitted once their activation has retired, so neither ever
  head-blocks the PE queue into a ScalarE stall. (Tile collapses
  dependencies into per-engine counter waits, so an exp's gate covers
  every PE instruction emitted before it - emission order IS the
  dependency structure.)
- PSUM: 2x3 banks QK-logit slots + 2x1 bank double-buffered PV
  accumulator. Loads are issued two heads ahead into 4 rotating
  SBUF slots (8 heads/rep: the 2-ahead load never collides with a
  live slot, including at the For_i wrap).

Measured (reps-differenced, per-rep): v7 269us -> v8 256-270us across
runs (device-state/thermal variance; best 255.9us) vs 324-361us for
the v2 baseline measured the same way in the same session. Rel err
5.0e-3 against the fp32 reference (gate 2e-2); simulated ScalarE
occupancy ~95% (251us busy of ~265us body). Under sustained load the
PE's P0 downclock (~2.0GHz) caps the floor near ~267us.

This container's walrus build rejects sync waits on Drain instructions
and allows at most one sync wait on any other instruction, while Tile
freely attaches several; _patch_tile_framework() + _split_sync_waits()
below rework the exit barrier and hoist excess waits onto injected NOPs.
"""
import sys

if '/opt/trn_rl_repo' not in sys.path:
    sys.path.insert(0, '/opt/trn_rl_repo')

import numpy as np
import ml_dtypes

import concourse.bass as bass
import concourse.tile as tile
from concourse import mybir
from concourse.vector_clock import ScopedClock

F32 = mybir.dt.float32
F16 = mybir.dt.bfloat16   # PE-native 16-bit dtype (78.6 TF/s); fp16 is not
EXP = mybir.ActivationFunctionType.Exp
BF16 = ml_dtypes.bfloat16

B, H, S, D = 4, 16, 2048, 64
N_CORES = 8
HEADS_PER_CORE = B * H // N_CORES


# ---------------------------------------------------------------------------
# Walrus compatibility patches (same as v1)
# ---------------------------------------------------------------------------
_patched = False
_split_counter = [0]


def _patched_multi_engine_barrier(self, engines):
    for e in engines:
        self.engines[e].drain(fusable=False)
    for inst in self._sem_only_all_engine_barrier_insts(f"aeb{self.next_id()}"):
        self.engines[inst.engine].add_instruction(inst)


def _patched_drain_and_barrier(self, tick_clock, wait_clock):
    nop_inst = self.nc.sync.nop(nofuse=True, hint="tile_exit_wait")
    wait_clock.add_sem_waits(
        nop_inst.ins, ScopedClock({None: tick_clock.global_clock})
    )
    self.nc.sync.drain()
    self.nc.all_engine_barrier()
    assert self.sems is not None
    popped = self.nc._tile_sem_poison_stack.pop()
    assert popped is self._sem_poison
    self.nc.clear_and_free_semaphores(list(self.sems.allocated().values()))
    self.nc.all_engine_barrier()


def _patch_tile_framework():
    global _patched
    if _patched:
        return
    bass.Bass.multi_engine_barrier = _patched_multi_engine_barrier
    tile.TileContext._drain_and_barrier = _patched_drain_and_barrier
    _patched = True


def _split_sync_waits(nc):
    """No instruction may carry more than the walrus-supported number of
    sync waits (0 for Drain, 1 otherwise); hoist the rest onto NOPs."""
    for f in nc.m.functions:
        for bb in f.blocks:
            insts = bb.instructions
            if not any(
                i.sync_info is not None
                and len(i.sync_info.on_wait) > (0 if i.opcode == "Drain" else 1)
                for i in insts
            ):
                continue
            out = []
            for inst in insts:
                si = inst.sync_info
                limit = 0 if inst.opcode == "Drain" else 1
                if si is not None and len(si.on_wait) > limit:
                    waits = list(si.on_wait)
                    keep, extra = waits[:limit], waits[limit:]
                    for w in extra:
                        _split_counter[0] += 1
                        nop = mybir.InstNoOp(
                            name=f"waitsplit-{_split_counter[0]}", ins=[], outs=[]
                        )
                        nop.engine = inst.engine
                        nop.sync_info = mybir.SyncInfo(on_wait=[w], on_update=[])
                        out.append(nop)
                    inst.sync_info = mybir.SyncInfo(
                        on_wait=keep, on_update=list(si.on_update)
                    )
                out.append(inst)
            bb.instructions = out


# ---------------------------------------------------------------------------
# Host-side input packing
# ---------------------------------------------------------------------------
def prep_inputs(q, k, v):
    """Pack full-shape fp32 q/k/v into per-core bf16 SBUF-layout tensors.

    qT: [hpc, 128, S]      qT[h, 64a+d, i]        = q[h, i, d]   (hi-dup)
    kT: [hpc, 128, S//2]   kT[h, 64a+d, 128cc+p]  = k[h, 256cc+128a+p, d]
    vt: [hpc, 128, NJ, 65] vt[h, p, c, d]         = v[h, 128c+p, d]; [...,64]=1
    """
    NJ = S // 128
    q = np.ascontiguousarray(np.asarray(q, np.float32)).reshape(B * H, S, D)
    k = np.ascontiguousarray(np.asarray(k, np.float32)).reshape(B * H, S, D)
    v = np.ascontiguousarray(np.asarray(v, np.float32)).reshape(B * H, S, D)

    qt = q.transpose(0, 2, 1)                                   # [64h?, D, S]
    qT = np.concatenate([qt, qt], axis=1).astype(BF16)          # [_, 128, S]

    kr = k.reshape(B * H, NJ // 2, 2, 128, D)                   # [_,cc,a,p,d]
    kT = np.ascontiguousarray(
        kr.transpose(0, 2, 4, 1, 3)                             # [_,a,d,cc,p]
    ).reshape(B * H, 128, (NJ // 2) * 128).astype(BF16)

    vr = v.reshape(B * H, NJ, 128, D).transpose(0, 2, 1, 3)     # [_,p,c,d]
    vt = np.empty((B * H, 128, NJ, D + 1), np.float32)
    vt[:, :, :, 0:D] = vr
    vt[:, :, :, D] = 1.0
    vt = vt.astype(BF16)

    hpc = HEADS_PER_CORE
    return [
        {"qT": qT[i * hpc:(i + 1) * hpc],
         "kT": kT[i * hpc:(i + 1) * hpc],
         "vt": vt[i * hpc:(i + 1) * hpc]}
        for i in range(N_CORES)
    ]


# ---------------------------------------------------------------------------
# Kernel builder
# ---------------------------------------------------------------------------
def build_nc(heads=HEADS_PER_CORE, s=S, reps=1):
    NJ = s // 128            # 16 j (k-row) chunks of 128
    NP = NJ // 2             # 8 chunk pairs
    IG = 512                 # i (q-row) group width
    NG = s // IG             # 4 groups
    NT = IG // 128           # 4 output tiles per group
    TICKS = NG * NP          # 32 ticks per head
    scale = D ** -0.5

    nc = bass.Bass(target_bir_lowering=False)
    qT_d = nc.dram_tensor("qT", [heads, 128, s], F16, kind="ExternalInput")
    kT_d = nc.dram_tensor("kT", [heads, 128, s // 2], F16, kind="ExternalInput")
    vt_d = nc.dram_tensor("vt", [heads, 128, NJ, D + 1], F16,
                          kind="ExternalInput")
    # Output is written d-major and UNNORMALIZED (65th row = softmax
    # denominator); the host divides and transposes back.
    o_d = nc.dram_tensor("o", [heads, D + 1, s], F32, kind="ExternalOutput")

    with tile.TileContext(nc) as tc:
        with (
            tc.tile_pool(name="perst", bufs=1) as perst,
            tc.tile_pool(name="ex1", bufs=6) as ex1,
            tc.tile_pool(name="osb", bufs=4) as osb,
            tc.tile_pool(name="qk3", bufs=1, space="PSUM") as qk3,
            tc.tile_pool(name="pvps", bufs=2, space="PSUM") as pvps,
        ):
            # Two persistent 3-bank QK-logit slots. Each holds up to three
            # 512-col "half-tick" thirds that are consumed by ONE FD-1536
            # activation, amortizing the ~220-cycle per-activation overhead
            # (176 instead of 256 activations per rep).
            pss = [qk3.tile([128, 3, IG], F32, tag=f"ps{i}", name=f"ps{i}")
                   for i in range(2)]

            # Persistent quad-buffered per-head tensors (slot h%4: with 8
            # heads per rep the 2-ahead loads never target a slot the
            # current or next head is reading, including at the wrap).
            qT = [perst.tile([128, s], F16, tag=f"qT{i}", name=f"qT{i}")
                  for i in range(4)]
            kT = [perst.tile([128, s // 2], F16, tag=f"kT{i}", name=f"kT{i}")
                  for i in range(4)]
            vt = [perst.tile([128, NJ, D + 1], F16, tag=f"vt{i}", name=f"vt{i}")
                  for i in range(4)]

            def load_ops(h):
                """DMA the pre-packed head h into slot h%3 (emitted two
                heads early; contiguous bf16 reads)."""
                sl = h % 4

                def load():
                    nc.sync.dma_start(out=qT[sl], in_=qT_d[h])
                    nc.sync.dma_start(out=kT[sl], in_=kT_d[h])
                    nc.sync.dma_start(out=vt[sl], in_=vt_d[h])
                return [(False, load)]

            def epi_ops(h, g, pv):
                """Epilogue closures for (head h, group g): evacuate pv to
                SBUF, DMA it out d-major (normalization happens on host)."""
                og = osb.tile([D + 1, IG], F32, tag="og")
                ops = []

                def evac():
                    nc.vector.tensor_copy(og, pv)
                ops.append((False, evac))

                def out_dma():
                    nc.sync.dma_start(
                        out=o_d[h, :, g * IG:(g + 1) * IG], in_=og)
                ops.append((False, out_dma))
                return ops

            # Per-head schedule: 64 half-ticks (g-major, chunk-minor), in
            # units of 3 (FD-1536 exp) with two 2-unit tails (FD-1024).
            UNIT_SIZES = [3] * 20 + [2, 2]

            def body():
                queue = []       # pending sprinkle closures [(is_pe, fn)]
                pv_tiles = {}    # (h, g) -> psum tile
                exp_pend = None  # (slot, size, [(h, g, c), ...]) awaiting exp
                pv_ready = []    # [(seq, (h, g, c), et_slice)] awaiting PV
                useq = [0]       # global unit counter

                def pop_sprinkles():
                    while queue:
                        queue.pop(0)[1]()

                def do_exp(slot, size, hgcs):
                    et = ex1.tile([128, 3, IG], F16, tag="et1")
                    nc.scalar.activation(
                        et[:, 0:size, :], pss[slot][:, 0:size, :],
                        EXP, scale=scale)
                    for idx, hgc in enumerate(hgcs):
                        pv_ready.append((useq[0], hgc, et[:, idx, :]))

                def emit_pv(hgc, et_slice):
                    h, g, c = hgc
                    if c == 0:
                        pv_tiles[(h, g)] = pvps.tile(
                            [D + 1, IG], F32, tag="pv", name="pv")
                    pv = pv_tiles[(h, g)]
                    nc.tensor.matmul(
                        pv, vt[h % 4][:, c, :], et_slice,
                        start=(c == 0), stop=(c == NJ - 1))
                    if c == NJ - 1:
                        queue.extend(epi_ops(h, g, pv))
                        del pv_tiles[(h, g)]

                for h in range(heads):
                    hts = [(g, c) for g in range(NG) for c in range(NJ)]
                    sl = h % 4
                    pos = 0
                    for u, size in enumerate(UNIT_SIZES):
                        if u == 0:
                            queue.extend(load_ops((h + 2) % heads))
                        slot = useq[0] % 2
                        cur = hts[pos:pos + size]
                        pos += size

                        # QK matmuls, one per half-tick, into slot thirds.
                        # 2-slot rotation: this unit's QKs wait only the exp
                        # issued two units back, whose window has passed.
                        for idx, (g, c) in enumerate(cur):
                            a = 64 * (c % 2)
                            nc.tensor.matmul(
                                pss[slot][:, idx, :],
                                kT[sl][a:a + 64,
                                       (c // 2) * 128:(c // 2) * 128 + 128],
                                qT[sl][a:a + 64, g * IG:(g + 1) * IG],
                                start=True, stop=True,
                                tile_position=(a, 0))

                        if exp_pend is not None:
                            do_exp(*exp_pend)   # exp for the previous unit
                        exp_pend = (slot, size, [(h, g, c) for g, c in cur])

                        pop_sprinkles()

                        # PV for half-ticks whose exp was issued in an
                        # earlier unit (keeps waiting PVs off the PE queue
                        # until their activation has retired).
                        while pv_ready and pv_ready[0][0] < useq[0]:
                            _, hgc, ets = pv_ready.pop(0)
                            emit_pv(hgc, ets)
                        useq[0] += 1

                # drain: last unit's exp, remaining PVs, sprinkles
                do_exp(*exp_pend)
                exp_pend = None
                while pv_ready:
                    _, hgc, ets = pv_ready.pop(0)
                    emit_pv(hgc, ets)
                while queue:
                    queue.pop(0)[1]()

            # Prologue: load heads 0+1 (one-time; outside For_i).
            for _, fn in load_ops(0) + load_ops(1):
                fn()

            if reps == 1:
                body()
            else:
                with tc.For_i(0, reps, 1):
                    body()

    _split_sync_waits(nc)
    return nc


_cached_nc = None


def _get_nc():
    global _cached_nc
    if _cached_nc is None:
        _patch_tile_framework()
        _cached_nc = build_nc()
    return _cached_nc


def kernel(q, k, v):
    """Full-shape attention: q/k/v [4, 16, 2048, 64] fp32 -> same shape."""
    from concourse.bass_utils import run_bass_kernel_spmd

    nc = _get_nc()
    in_maps = prep_inputs(q, k, v)
    res = run_bass_kernel_spmd(nc, in_maps, core_ids=list(range(N_CORES)))
    out = np.concatenate([res.results[i]["o"] for i in range(N_CORES)], axis=0)
    # out: [B*H, D+1, S] unnormalized, d-major; row D is the softmax denom.
    out = out[:, 0:D, :] / out[:, D:D + 1, :]
    return np.ascontiguousarray(out.transpose(0, 2, 1)).reshape(B, H, S, D)
